# revision 20
# baseline (speedup 1.0000x reference)
"""Trainium2 Bass kernel for the DifferentiableTVLayer PDHG solve.

Problem: anisotropic weighted-TV prox via Chambolle-Pock on 8 images of
256x256 (fp32). Sharding: pure data parallel, 1 image per NeuronCore.

The reference runs 200 iterations but is itself ~0.006 rel away from the
converged solution; 120 iterations of the same scheme land within ~0.010
rel of the reference output (tolerance 2e-2), so we run 120.

Per-core layout ("layout A"): tiles [128, 512] where
    tile[p, c*256 + h] = X[h, w],  w = c*128 + p,  c in {0,1}.
W-direction (partition) derivatives are PE matmuls against +-1 operator
matrices; H-direction (free dim) derivatives are PE matmuls against
identity with column-shifted access patterns, accumulated in PSUM.

All PE weights are exactly-representable fp16 constants (+-1 or +-KKH);
all exact scale factors (C1, 2C1-1, 2.0) ride as fp32 immediates in DVE
scalar_tensor_tensor ops, so fp16 only contributes state-quantization
noise (~2e-4), damped by the C1-contraction of the iteration.

Per iteration (state: U fp32; VB, P, Q fp16; bounds LX, LY fp16):
    PSq = I@Qc + Ly@VB + Ey@VB[hi]            (PE, = q + sigma*dy(ubar))
    PSp = I@Pc + (-I)@VB + I@VB<<1            (PE, = p + sigma*dx(ubar))
    PSu = I@CFP + KKH*(dxT(Pn) + dyT(Qn))     (PE, scaled primal residual)
    Pn  = clip(PSp, +-LX); Qn = clip(PSq, +-LY)   (DVE fused custom op)
    Un  = C1*Uc + PSu                          (DVE STT)
    VBn = 2*PSu + A2,  A2 = (2C1-1)*Uc         (DVE STT; A2 on ScalarE)
"""

import numpy as np

import concourse.bass as bass
import concourse.mybir as mybir
from concourse.tile import TileContext

TAU0 = 0.5                  # accelerated CP (Chambolle-Pock Alg.2, mu-strongly convex)
SIGMA0 = 1.0 / (8.0 * TAU0)  # tau*sigma*L^2 = 1 boundary
GAMMA = 0.5
KH = 0.25                    # fixed fp16-exact PE weight for the PSu terms
B, H, W = 8, 256, 256
P, NCH = 128, 2
FREE = NCH * H  # 512
N_ITERS = 44


def _schedule(n):
    """Per-iteration (tau_i, sigma_i, theta_i), plus sigma_{n} lookahead."""
    taus, sigs, thetas = [], [], []
    tau, sig = TAU0, SIGMA0
    for _ in range(n + 1):
        th = 1.0 / float(np.sqrt(1.0 + 2.0 * GAMMA * tau))
        taus.append(tau)
        sigs.append(sig)
        thetas.append(th)
        tau, sig = tau * th, sig / th
    return taus, sigs, thetas

F32 = mybir.dt.float32
F16 = mybir.dt.float16
AOP = mybir.AluOpType


# ---------------------------------------------------------------- host layout
def _to_layout_a(x):
    """[H, W] -> [128, 512]: out[p, c*256+h] = x[h, c*128+p]."""
    return np.ascontiguousarray(
        x.T.reshape(NCH, P, H).transpose(1, 0, 2).reshape(P, FREE)
    )


def _from_layout_a(t):
    return np.ascontiguousarray(
        t.reshape(P, NCH, H).transpose(1, 0, 2).reshape(W, H).T
    )


def _make_matrices():
    """lhsT operator matrices [k, m]: out[m] = sum_k lhsT[k,m] rhs[k]."""
    kk = np.float32(np.float16(KH))
    I = np.eye(P, dtype=np.float32)
    Ly = np.zeros((P, P), np.float32)
    for m in range(P):
        Ly[m, m] = -1.0
        if m + 1 < P:
            Ly[m + 1, m] = 1.0
    Ey = np.zeros((P, P), np.float32)
    Ey[0, 127] = 1.0
    KyT = np.zeros((P, P), np.float32)
    for m in range(P):
        KyT[m, m] = kk
        if m - 1 >= 0:
            KyT[m - 1, m] = -kk
    KeT = np.zeros((P, P), np.float32)
    KeT[127, 0] = -kk
    mats = {"mID": I, "mLy": Ly, "mEy": Ey, "mNI": -I, "mKp": kk * I,
            "mKn": -kk * I, "mKyT": KyT, "mKeT": KeT}
    return {k: v.astype(np.float16) for k, v in mats.items()}


def _per_core_inputs(f_img, lam_img):
    fa = _to_layout_a(np.asarray(f_img, np.float32))
    u0 = fa.astype(np.float32)                      # W = u
    vb0 = (SIGMA0 * fa).astype(np.float16)
    cfp = (KH * fa).astype(np.float16)

    lamx = np.concatenate([lam_img[1:, :], np.zeros((1, W), np.float32)])
    lx3 = _to_layout_a(lamx).reshape(P, NCH, H).copy()
    lx3[:, :, 255] = 0.0
    lx = lx3.reshape(P, FREE).astype(np.float16)

    lamy = np.concatenate([lam_img[:, 1:], np.zeros((H, 1), np.float32)], axis=1)
    ly = _to_layout_a(lamy).astype(np.float16)  # (c=1,p=127) col already zero

    return {
        "u0": np.ascontiguousarray(u0),
        "vb0": np.ascontiguousarray(vb0),
        "cfp": np.ascontiguousarray(cfp),
        "lx": np.ascontiguousarray(lx),
        "ly": np.ascontiguousarray(ly),
    }


# ---------------------------------------------------------------- custom op
def _register_clip_op():
    """out = clip(in0, -in1, +in1) as a single DVE instruction."""
    from concourse import dve_ops
    from concourse.dve_spec import Spec, Src0, Src1, maxx, minn, lower
    from concourse.dve_uop import DveOpSpec

    for op in dve_ops.OPS:
        if op.name == "TV_CLIP2_ANT":
            return op
    spec = Spec(
        body=minn(maxx(Src0, -Src1), Src1),
        reference=lambda in0, in1, s0, s1, imm2: np.minimum(
            np.maximum(in0, -in1), in1
        ).astype(np.float32),
    )
    op = dve_ops.DveOp("TV_CLIP2_ANT", spec, subdim=False, uops_sha={})
    dve_ops.OPS.append(op)
    dve_ops.CUSTOM_DVE_SPECS[op.name] = spec
    dve_ops._SUB_OPCODE_FOR_NAME[op.name] = (
        max(dve_ops._SUB_OPCODE_FOR_NAME.values()) + 1
    )
    for ver in ("v3", "v4"):
        try:
            s = DveOpSpec(
                name=op.name,
                opcode=dve_ops.get_dve_sub_opcode(op.name),
                uops=lower(spec, ver=ver),
                rd1_en=True,
            )
            op.uops_sha[ver] = s.sha(ver)
        except Exception:
            pass
    return op


CLIP_OP = _register_clip_op()


# ---------------------------------------------------------------- bass build
def split_excess_waits(nc, max_waits=1):
    """This neuronxcc/walrus build encodes at most ONE sync wait per
    instruction; split the excess onto NoOp carriers on the same engine."""
    nsplit = 0
    for f in nc.m.functions:
        for bb in f.blocks:
            il = bb.instructions
            out = []
            for inst in il:
                si = inst.sync_info
                waits = list(si.on_wait) if si and si.on_wait else []
                k = 0
                while len(waits) > max_waits:
                    head, waits = waits[:max_waits], waits[max_waits:]
                    out.append(
                        mybir.InstNoOp(
                            name=f"{inst.name}-waitsplit{k}",
                            engine=inst.engine,
                            ins=[],
                            outs=[],
                            sync_info=mybir.SyncInfo(on_wait=head, on_update=[]),
                        )
                    )
                    k += 1
                    nsplit += 1
                if k:
                    inst.sync_info = mybir.SyncInfo(
                        on_wait=waits,
                        on_update=list(si.on_update) if si.on_update else [],
                    )
                out.append(inst)
            il[:] = out
    return nsplit


USE_FUSED_CLIP = False
FUSE_Q = False


def build_nc(n_iters=N_ITERS, split=True, fused_clip=None):
    if fused_clip is None:
        fused_clip = USE_FUSED_CLIP
    nc = bass.Bass(trn_type="TRN2")

    d_in = {"u0": nc.dram_tensor("u0", [P, FREE], F32, kind="ExternalInput")}
    for name in ("vb0", "cfp", "lx", "ly"):
        d_in[name] = nc.dram_tensor(name, [P, FREE], F16, kind="ExternalInput")
    d_out = nc.dram_tensor("out", [P, FREE], F32, kind="ExternalOutput")
    d_mats = {
        name: nc.inline_tensor(data, name=name)
        for name, data in _make_matrices().items()
    }

    taus, sigs, thetas = _schedule(n_iters)
    with TileContext(nc) as tc:
        with (
            tc.tile_pool(name="state", bufs=1) as state,
            tc.tile_pool(name="scratch", bufs=3) as scratch,
            tc.tile_pool(name="psum", bufs=2, space="PSUM") as psum,
        ):
            # persistent tiles
            Us = [state.tile([P, FREE], F32, name=f"U{i}") for i in range(2)]
            VBs = [state.tile([P, FREE], F16, name=f"VB{i}") for i in range(2)]
            Ps = [state.tile([P, FREE + 2], F16, name=f"Pd{i}") for i in range(2)]
            Qs = [state.tile([P, FREE], F16, name=f"Qd{i}") for i in range(2)]
            LX = state.tile([P, FREE], F16, name="LX")
            LY = state.tile([P, FREE], F16, name="LY")
            NLX = state.tile([P, FREE], F16, name="NLX")
            NLY = state.tile([P, FREE], F16, name="NLY")
            CF = state.tile([P, FREE], F16, name="CF")
            mats = {
                name: state.tile([P, P], F16, name=f"t_{name}") for name in d_mats
            }

            # ---- setup
            nc.sync.dma_start(out=Us[0], in_=d_in["u0"].ap())
            nc.gpsimd.dma_start(out=VBs[0], in_=d_in["vb0"].ap())
            nc.scalar.dma_start(out=CF, in_=d_in["cfp"].ap())
            nc.gpsimd.dma_start(out=LX, in_=d_in["lx"].ap())
            nc.sync.dma_start(out=LY, in_=d_in["ly"].ap())
            for i, name in enumerate(d_mats):
                dma = (nc.gpsimd, nc.sync, nc.scalar)[i % 3]
                dma.dma_start(out=mats[name], in_=d_mats[name].ap())
            nc.vector.memset(Ps[0].bitcast(F32), 0.0)
            nc.vector.memset(Ps[1].bitcast(F32), 0.0)
            nc.vector.memset(Qs[0].bitcast(F32), 0.0)
            if not fused_clip:
                nc.scalar.mul(NLX, LX, -1.0)
                nc.scalar.mul(NLY, LY, -1.0)

            def clip2(out, psrc, L, NL, tag, fuse=None):
                if fused_clip if fuse is None else fuse:
                    L3 = L.rearrange("p (s n) -> p s n", s=1)
                    nc.vector._custom_dve(CLIP_OP, out=out, in0=psrc, in1=L3)
                else:
                    t = scratch.tile([P, FREE], F16, name=f"cl_{tag}", tag=f"cl_{tag}")
                    nc.vector.tensor_tensor(t, psrc, L, AOP.min)
                    nc.vector.tensor_tensor(out, t, NL, AOP.max)

            def mm(out, lhsT, rhs, start, stop):
                nc.tensor.matmul(
                    out, lhsT, rhs, start=start, stop=stop,
                    skip_group_check=True,
                )

            for i in range(n_iters):
                a, b = i % 2, (i + 1) % 2
                Uc, Un = Us[a], Us[b]
                VBc, VBn = VBs[a], VBs[b]
                Pc, Pn = Ps[a], Ps[b]
                Qc, Qn = Qs[a], Qs[b]

                PSq = psum.tile([P, FREE], F32, name="PSq", tag="PSq")
                PSp = psum.tile([P, FREE], F32, name="PSp", tag="PSp")
                PSu = psum.tile([P, FREE], F32, name="PSu", tag="PSu")

                # early matmuls (deps from mid-previous-iteration)
                mm(PSq, mats["mID"], Qc, start=True, stop=False)
                mm(PSp, mats["mID"], Pc[:, 1 : FREE + 1], start=True, stop=False)
                mm(PSu, mats["mID"], CF, start=True, stop=False)

                # on VBc: p-path first (it gates the longer tail)
                mm(PSp, mats["mNI"], VBc, start=False, stop=False)
                mm(PSp[:, 0 : H - 1], mats["mID"], VBc[:, 1:H], start=False,
                   stop=False)
                mm(PSp[:, H : FREE - 1], mats["mID"], VBc[:, H + 1 : FREE],
                   start=False, stop=True)
                mm(PSq, mats["mLy"], VBc, start=False, stop=False)
                mm(PSq[:, 0:H], mats["mEy"], VBc[:, H:FREE], start=False,
                   stop=True)

                tau, th = taus[i], thetas[i]
                sig1 = sigs[i + 1]
                c1 = 1.0 / (1.0 + tau)
                s1 = float(c1 * tau / KH)
                s2 = float(sig1 * (1.0 + th) * c1 * tau / KH)
                a1 = float(c1)
                a2 = float(sig1 * ((1.0 + th) * c1 - th))

                # dual clips: P straight from PSUM; Q via ScalarE fp16 cast
                # so both Q ops run in DVE 2x mode
                clip2(Pn[:, 1 : FREE + 1], PSp, LX, NLX, "p")
                Q16 = scratch.tile([P, FREE], F16, name="Q16", tag="Q16")
                nc.scalar.copy(Q16, PSq)
                clip2(Qn, Q16, LY, NLY, "q")
                A2 = scratch.tile([P, FREE], F32, name="A2", tag="A2")
                A1 = scratch.tile([P, FREE], F32, name="A1", tag="A1")
                nc.gpsimd.tensor_scalar_mul(A2, Uc, a2)
                nc.gpsimd.tensor_scalar_mul(A1, Uc, a1)

                # primal tail
                mm(PSu, mats["mKp"], Pn[:, 1 : FREE + 1], start=False,
                   stop=False)
                mm(PSu, mats["mKn"], Pn[:, 0:FREE], start=False, stop=False)
                mm(PSu, mats["mKyT"], Qn, start=False, stop=False)
                mm(PSu[:, H:FREE], mats["mKeT"], Qn[:, 0:H], start=False,
                   stop=True)

                nc.vector.scalar_tensor_tensor(
                    out=VBn, in0=PSu, scalar=s2, in1=A2,
                    op0=AOP.mult, op1=AOP.add,
                )
                nc.vector.scalar_tensor_tensor(
                    out=Un, in0=PSu, scalar=s1, in1=A1,
                    op0=AOP.mult, op1=AOP.add,
                )

            Ufin = Us[n_iters % 2]
            nc.sync.dma_start(out=d_out.ap(), in_=Ufin)

    nc.finalize()
    if split:
        split_excess_waits(nc)
    return nc


_NC_CACHE = {}


def _get_nc(n_iters=N_ITERS):
    key = n_iters
    if key not in _NC_CACHE:
        _NC_CACHE[key] = build_nc(n_iters)
    return _NC_CACHE[key]


def kernel(f, lam):
    from concourse.bass_utils import run_bass_kernel_spmd

    f = np.asarray(f, dtype=np.float32)
    lam = np.asarray(lam, dtype=np.float32)
    nc = _get_nc()
    in_maps = [_per_core_inputs(f[b], lam[b]) for b in range(B)]
    res = run_bass_kernel_spmd(nc, in_maps, core_ids=list(range(B)))
    return np.stack([_from_layout_a(res.results[b]["out"]) for b in range(B)])


if __name__ == "__main__":
    import sys

    if "--build" in sys.argv:
        import time

        t0 = time.time()
        nc = build_nc(int(sys.argv[sys.argv.index("--build") + 1])
                      if len(sys.argv) > 2 else N_ITERS)
        print(f"build ok in {time.time()-t0:.1f}s")


# revision 21
# speedup vs baseline: 3.5357x; 3.5357x over previous
"""Trainium2 Bass kernel for the DifferentiableTVLayer PDHG solve.

Problem: anisotropic weighted-TV prox via Chambolle-Pock on 8 images of
256x256 (fp32). Sharding: pure data parallel, 1 image per NeuronCore.

The reference runs 200 iterations but is itself ~0.006 rel away from the
converged solution; 120 iterations of the same scheme land within ~0.010
rel of the reference output (tolerance 2e-2), so we run 120.

Per-core layout ("layout A"): tiles [128, 512] where
    tile[p, c*256 + h] = X[h, w],  w = c*128 + p,  c in {0,1}.
W-direction (partition) derivatives are PE matmuls against +-1 operator
matrices; H-direction (free dim) derivatives are PE matmuls against
identity with column-shifted access patterns, accumulated in PSUM.

All PE weights are exactly-representable fp16 constants (+-1 or +-KKH);
all exact scale factors (C1, 2C1-1, 2.0) ride as fp32 immediates in DVE
scalar_tensor_tensor ops, so fp16 only contributes state-quantization
noise (~2e-4), damped by the C1-contraction of the iteration.

Per iteration (state: U fp32; VB, P, Q fp16; bounds LX, LY fp16):
    PSq = I@Qc + Ly@VB + Ey@VB[hi]            (PE, = q + sigma*dy(ubar))
    PSp = I@Pc + (-I)@VB + I@VB<<1            (PE, = p + sigma*dx(ubar))
    PSu = I@CFP + KKH*(dxT(Pn) + dyT(Qn))     (PE, scaled primal residual)
    Pn  = clip(PSp, +-LX); Qn = clip(PSq, +-LY)   (DVE fused custom op)
    Un  = C1*Uc + PSu                          (DVE STT)
    VBn = 2*PSu + A2,  A2 = (2C1-1)*Uc         (DVE STT; A2 on ScalarE)
"""

import numpy as np

import concourse.bass as bass
import concourse.mybir as mybir
from concourse.tile import TileContext

TAU0 = 0.5                  # accelerated CP (Chambolle-Pock Alg.2, mu-strongly convex)
SIGMA0 = 1.0 / (8.0 * TAU0)  # tau*sigma*L^2 = 1 boundary
GAMMA = 0.5
KH = 0.25                    # fixed fp16-exact PE weight for the PSu terms
B, H, W = 8, 256, 256
P, NCH = 128, 2
FREE = NCH * H  # 512
N_ITERS = 44


def _schedule(n):
    """Per-iteration (tau_i, sigma_i, theta_i), plus sigma_{n} lookahead."""
    taus, sigs, thetas = [], [], []
    tau, sig = TAU0, SIGMA0
    for _ in range(n + 1):
        th = 1.0 / float(np.sqrt(1.0 + 2.0 * GAMMA * tau))
        taus.append(tau)
        sigs.append(sig)
        thetas.append(th)
        tau, sig = tau * th, sig / th
    return taus, sigs, thetas

F32 = mybir.dt.float32
F16 = mybir.dt.float16
AOP = mybir.AluOpType


# ---------------------------------------------------------------- host layout
def _to_layout_a(x):
    """[H, W] -> [128, 512]: out[p, c*256+h] = x[h, c*128+p]."""
    return np.ascontiguousarray(
        x.T.reshape(NCH, P, H).transpose(1, 0, 2).reshape(P, FREE)
    )


def _from_layout_a(t):
    return np.ascontiguousarray(
        t.reshape(P, NCH, H).transpose(1, 0, 2).reshape(W, H).T
    )


def _make_matrices():
    """lhsT operator matrices [k, m]: out[m] = sum_k lhsT[k,m] rhs[k]."""
    kk = np.float32(np.float16(KH))
    I = np.eye(P, dtype=np.float32)
    Ly = np.zeros((P, P), np.float32)
    for m in range(P):
        Ly[m, m] = -1.0
        if m + 1 < P:
            Ly[m + 1, m] = 1.0
    Ey = np.zeros((P, P), np.float32)
    Ey[0, 127] = 1.0
    KyT = np.zeros((P, P), np.float32)
    for m in range(P):
        KyT[m, m] = kk
        if m - 1 >= 0:
            KyT[m - 1, m] = -kk
    KeT = np.zeros((P, P), np.float32)
    KeT[127, 0] = -kk
    mats = {"mID": I, "mLy": Ly, "mEy": Ey, "mNI": -I, "mKp": kk * I,
            "mKn": -kk * I, "mKyT": KyT, "mKeT": KeT}
    return {k: v.astype(np.float16) for k, v in mats.items()}


def _per_core_inputs(f_img, lam_img):
    fa = _to_layout_a(np.asarray(f_img, np.float32))
    u0 = fa.astype(np.float32)                      # W = u
    vb0 = (SIGMA0 * fa).astype(np.float16)
    cfp = (KH * fa).astype(np.float16)

    lamx = np.concatenate([lam_img[1:, :], np.zeros((1, W), np.float32)])
    lx3 = _to_layout_a(lamx).reshape(P, NCH, H).copy()
    lx3[:, :, 255] = 0.0
    lx = lx3.reshape(P, FREE).astype(np.float16)

    lamy = np.concatenate([lam_img[:, 1:], np.zeros((H, 1), np.float32)], axis=1)
    ly = _to_layout_a(lamy).astype(np.float16)  # (c=1,p=127) col already zero

    return {
        "u0": np.ascontiguousarray(u0),
        "vb0": np.ascontiguousarray(vb0),
        "cfp": np.ascontiguousarray(cfp),
        "lx": np.ascontiguousarray(lx),
        "ly": np.ascontiguousarray(ly),
    }


# ---------------------------------------------------------------- custom op
def _register_clip_op():
    """out = clip(in0, -in1, +in1) as a single DVE instruction."""
    from concourse import dve_ops
    from concourse.dve_spec import Spec, Src0, Src1, maxx, minn, lower
    from concourse.dve_uop import DveOpSpec

    for op in dve_ops.OPS:
        if op.name == "TV_CLIP2_ANT":
            return op
    spec = Spec(
        body=minn(maxx(Src0, -Src1), Src1),
        reference=lambda in0, in1, s0, s1, imm2: np.minimum(
            np.maximum(in0, -in1), in1
        ).astype(np.float32),
    )
    op = dve_ops.DveOp("TV_CLIP2_ANT", spec, subdim=False, uops_sha={})
    dve_ops.OPS.append(op)
    dve_ops.CUSTOM_DVE_SPECS[op.name] = spec
    dve_ops._SUB_OPCODE_FOR_NAME[op.name] = (
        max(dve_ops._SUB_OPCODE_FOR_NAME.values()) + 1
    )
    for ver in ("v3", "v4"):
        try:
            s = DveOpSpec(
                name=op.name,
                opcode=dve_ops.get_dve_sub_opcode(op.name),
                uops=lower(spec, ver=ver),
                rd1_en=True,
            )
            op.uops_sha[ver] = s.sha(ver)
        except Exception:
            pass
    return op


CLIP_OP = _register_clip_op()


# ---------------------------------------------------------------- bass build
def split_excess_waits(nc, max_waits=1):
    """This neuronxcc/walrus build encodes at most ONE sync wait per
    instruction; split the excess onto NoOp carriers on the same engine."""
    nsplit = 0
    for f in nc.m.functions:
        for bb in f.blocks:
            il = bb.instructions
            out = []
            for inst in il:
                si = inst.sync_info
                waits = list(si.on_wait) if si and si.on_wait else []
                k = 0
                while len(waits) > max_waits:
                    head, waits = waits[:max_waits], waits[max_waits:]
                    out.append(
                        mybir.InstNoOp(
                            name=f"{inst.name}-waitsplit{k}",
                            engine=inst.engine,
                            ins=[],
                            outs=[],
                            sync_info=mybir.SyncInfo(on_wait=head, on_update=[]),
                        )
                    )
                    k += 1
                    nsplit += 1
                if k:
                    inst.sync_info = mybir.SyncInfo(
                        on_wait=waits,
                        on_update=list(si.on_update) if si.on_update else [],
                    )
                out.append(inst)
            il[:] = out
    return nsplit


USE_FUSED_CLIP = False
FUSE_Q = False


def build_nc(n_iters=N_ITERS, split=True, fused_clip=None):
    if fused_clip is None:
        fused_clip = USE_FUSED_CLIP
    nc = bass.Bass(trn_type="TRN2")

    d_in = {"u0": nc.dram_tensor("u0", [P, FREE], F32, kind="ExternalInput")}
    for name in ("vb0", "cfp", "lx", "ly"):
        d_in[name] = nc.dram_tensor(name, [P, FREE], F16, kind="ExternalInput")
    d_out = nc.dram_tensor("out", [P, FREE], F32, kind="ExternalOutput")
    d_mats = {
        name: nc.inline_tensor(data, name=name)
        for name, data in _make_matrices().items()
    }

    taus, sigs, thetas = _schedule(n_iters)
    with TileContext(nc) as tc:
        with (
            tc.tile_pool(name="state", bufs=1) as state,
            tc.tile_pool(name="scratch", bufs=3) as scratch,
            tc.tile_pool(name="psum", bufs=2, space="PSUM") as psum,
        ):
            # persistent tiles
            Us = [state.tile([P, FREE], F32, name=f"U{i}") for i in range(2)]
            VBs = [state.tile([P, FREE], F16, name=f"VB{i}") for i in range(2)]
            Ps = [state.tile([P, FREE + 2], F16, name=f"Pd{i}") for i in range(2)]
            Qs = [state.tile([P, FREE], F16, name=f"Qd{i}") for i in range(2)]
            LX = state.tile([P, FREE], F16, name="LX")
            LY = state.tile([P, FREE], F16, name="LY")
            NLX = state.tile([P, FREE], F16, name="NLX")
            NLY = state.tile([P, FREE], F16, name="NLY")
            CF = state.tile([P, FREE], F16, name="CF")
            mats = {
                name: state.tile([P, P], F16, name=f"t_{name}") for name in d_mats
            }

            # ---- setup
            nc.sync.dma_start(out=Us[0], in_=d_in["u0"].ap())
            nc.gpsimd.dma_start(out=VBs[0], in_=d_in["vb0"].ap())
            nc.scalar.dma_start(out=CF, in_=d_in["cfp"].ap())
            nc.gpsimd.dma_start(out=LX, in_=d_in["lx"].ap())
            nc.sync.dma_start(out=LY, in_=d_in["ly"].ap())
            for i, name in enumerate(d_mats):
                dma = (nc.gpsimd, nc.sync, nc.scalar)[i % 3]
                dma.dma_start(out=mats[name], in_=d_mats[name].ap())
            nc.vector.memset(Ps[0].bitcast(F32), 0.0)
            nc.vector.memset(Ps[1].bitcast(F32), 0.0)
            nc.vector.memset(Qs[0].bitcast(F32), 0.0)
            if not fused_clip:
                nc.scalar.mul(NLX, LX, -1.0)
                nc.scalar.mul(NLY, LY, -1.0)

            def clip2(out, psrc, L, NL, tag, fuse=None):
                if fused_clip if fuse is None else fuse:
                    L3 = L.rearrange("p (s n) -> p s n", s=1)
                    nc.vector._custom_dve(CLIP_OP, out=out, in0=psrc, in1=L3)
                else:
                    t = scratch.tile([P, FREE], F16, name=f"cl_{tag}", tag=f"cl_{tag}")
                    nc.vector.tensor_tensor(t, psrc, L, AOP.min)
                    nc.vector.tensor_tensor(out, t, NL, AOP.max)

            def mm(out, lhsT, rhs, start, stop):
                nc.tensor.matmul(
                    out, lhsT, rhs, start=start, stop=stop,
                    skip_group_check=True,
                )

            for i in range(n_iters):
                a, b = i % 2, (i + 1) % 2
                Uc, Un = Us[a], Us[b]
                VBc, VBn = VBs[a], VBs[b]
                Pc, Pn = Ps[a], Ps[b]
                Qc, Qn = Qs[a], Qs[b]

                PSq = psum.tile([P, FREE], F32, name="PSq", tag="PSq")
                PSp = psum.tile([P, FREE], F32, name="PSp", tag="PSp")
                PSu = psum.tile([P, FREE], F32, name="PSu", tag="PSu")

                # early matmuls (deps from mid-previous-iteration)
                mm(PSq, mats["mID"], Qc, start=True, stop=False)
                mm(PSp, mats["mID"], Pc[:, 1 : FREE + 1], start=True, stop=False)
                mm(PSu, mats["mID"], CF, start=True, stop=False)

                # on VBc: q-path first (cast readiness beats A2 on ScalarE)
                mm(PSq, mats["mLy"], VBc, start=False, stop=False)
                mm(PSq[:, 0:H], mats["mEy"], VBc[:, H:FREE], start=False,
                   stop=True)
                mm(PSp, mats["mNI"], VBc, start=False, stop=False)
                mm(PSp[:, 0 : H - 1], mats["mID"], VBc[:, 1:H], start=False,
                   stop=False)
                mm(PSp[:, H : FREE - 1], mats["mID"], VBc[:, H + 1 : FREE],
                   start=False, stop=True)

                tau, th = taus[i], thetas[i]
                sig1 = sigs[i + 1]
                c1 = 1.0 / (1.0 + tau)
                s1 = float(c1 * tau / KH)
                s2 = float(sig1 * (1.0 + th) * c1 * tau / KH)
                a1 = float(c1)
                a2 = float(sig1 * ((1.0 + th) * c1 - th))

                # dual clips: P straight from PSUM; Q via ScalarE fp16 cast
                # so both Q ops run in DVE 2x mode
                clip2(Pn[:, 1 : FREE + 1], PSp, LX, NLX, "p")
                Q16 = scratch.tile([P, FREE], F16, name="Q16", tag="Q16")
                nc.scalar.copy(Q16, PSq)
                clip2(Qn, Q16, LY, NLY, "q")
                A2 = scratch.tile([P, FREE], F32, name="A2", tag="A2")
                A1 = scratch.tile([P, FREE], F32, name="A1", tag="A1")
                nc.scalar.mul(A2, Uc, a2)
                nc.scalar.mul(A1, Uc, a1)

                # primal tail
                mm(PSu, mats["mKp"], Pn[:, 1 : FREE + 1], start=False,
                   stop=False)
                mm(PSu, mats["mKn"], Pn[:, 0:FREE], start=False, stop=False)
                mm(PSu, mats["mKyT"], Qn, start=False, stop=False)
                mm(PSu[:, H:FREE], mats["mKeT"], Qn[:, 0:H], start=False,
                   stop=True)

                nc.vector.scalar_tensor_tensor(
                    out=VBn, in0=PSu, scalar=s2, in1=A2,
                    op0=AOP.mult, op1=AOP.add,
                )
                nc.vector.scalar_tensor_tensor(
                    out=Un, in0=PSu, scalar=s1, in1=A1,
                    op0=AOP.mult, op1=AOP.add,
                )

            Ufin = Us[n_iters % 2]
            nc.sync.dma_start(out=d_out.ap(), in_=Ufin)

    nc.finalize()
    if split:
        split_excess_waits(nc)
    return nc


_NC_CACHE = {}


def _get_nc(n_iters=N_ITERS):
    key = n_iters
    if key not in _NC_CACHE:
        _NC_CACHE[key] = build_nc(n_iters)
    return _NC_CACHE[key]


def kernel(f, lam):
    from concourse.bass_utils import run_bass_kernel_spmd

    f = np.asarray(f, dtype=np.float32)
    lam = np.asarray(lam, dtype=np.float32)
    nc = _get_nc()
    in_maps = [_per_core_inputs(f[b], lam[b]) for b in range(B)]
    res = run_bass_kernel_spmd(nc, in_maps, core_ids=list(range(B)))
    return np.stack([_from_layout_a(res.results[b]["out"]) for b in range(B)])


if __name__ == "__main__":
    import sys

    if "--build" in sys.argv:
        import time

        t0 = time.time()
        nc = build_nc(int(sys.argv[sys.argv.index("--build") + 1])
                      if len(sys.argv) > 2 else N_ITERS)
        print(f"build ok in {time.time()-t0:.1f}s")


# revision 22
# speedup vs baseline: 3.7962x; 1.0737x over previous
"""Trainium2 Bass kernel for the DifferentiableTVLayer PDHG solve.

Problem: anisotropic weighted-TV prox via Chambolle-Pock on 8 images of
256x256 (fp32). Sharding: pure data parallel, 1 image per NeuronCore.

The reference runs 200 iterations but is itself ~0.006 rel away from the
converged solution; 120 iterations of the same scheme land within ~0.010
rel of the reference output (tolerance 2e-2), so we run 120.

Per-core layout ("layout A"): tiles [128, 512] where
    tile[p, c*256 + h] = X[h, w],  w = c*128 + p,  c in {0,1}.
W-direction (partition) derivatives are PE matmuls against +-1 operator
matrices; H-direction (free dim) derivatives are PE matmuls against
identity with column-shifted access patterns, accumulated in PSUM.

All PE weights are exactly-representable fp16 constants (+-1 or +-KKH);
all exact scale factors (C1, 2C1-1, 2.0) ride as fp32 immediates in DVE
scalar_tensor_tensor ops, so fp16 only contributes state-quantization
noise (~2e-4), damped by the C1-contraction of the iteration.

Per iteration (state: U fp32; VB, P, Q fp16; bounds LX, LY fp16):
    PSq = I@Qc + Ly@VB + Ey@VB[hi]            (PE, = q + sigma*dy(ubar))
    PSp = I@Pc + (-I)@VB + I@VB<<1            (PE, = p + sigma*dx(ubar))
    PSu = I@CFP + KKH*(dxT(Pn) + dyT(Qn))     (PE, scaled primal residual)
    Pn  = clip(PSp, +-LX); Qn = clip(PSq, +-LY)   (DVE fused custom op)
    Un  = C1*Uc + PSu                          (DVE STT)
    VBn = 2*PSu + A2,  A2 = (2C1-1)*Uc         (DVE STT; A2 on ScalarE)
"""

import numpy as np

import concourse.bass as bass
import concourse.mybir as mybir
from concourse.tile import TileContext

TAU0 = 0.5                  # accelerated CP (Chambolle-Pock Alg.2, mu-strongly convex)
SIGMA0 = 1.0 / (8.0 * TAU0)  # tau*sigma*L^2 = 1 boundary
GAMMA = 0.5
KH = 0.25                    # fixed fp16-exact PE weight for the PSu terms
B, H, W = 8, 256, 256
P, NCH = 128, 2
FREE = NCH * H  # 512
N_ITERS = 40


def _schedule(n):
    """Per-iteration (tau_i, sigma_i, theta_i), plus sigma_{n} lookahead."""
    taus, sigs, thetas = [], [], []
    tau, sig = TAU0, SIGMA0
    for _ in range(n + 1):
        th = 1.0 / float(np.sqrt(1.0 + 2.0 * GAMMA * tau))
        taus.append(tau)
        sigs.append(sig)
        thetas.append(th)
        tau, sig = tau * th, sig / th
    return taus, sigs, thetas

F32 = mybir.dt.float32
F16 = mybir.dt.float16
AOP = mybir.AluOpType


# ---------------------------------------------------------------- host layout
def _to_layout_a(x):
    """[H, W] -> [128, 512]: out[p, c*256+h] = x[h, c*128+p]."""
    return np.ascontiguousarray(
        x.T.reshape(NCH, P, H).transpose(1, 0, 2).reshape(P, FREE)
    )


def _from_layout_a(t):
    return np.ascontiguousarray(
        t.reshape(P, NCH, H).transpose(1, 0, 2).reshape(W, H).T
    )


def _make_matrices():
    """lhsT operator matrices [k, m]: out[m] = sum_k lhsT[k,m] rhs[k]."""
    kk = np.float32(np.float16(KH))
    I = np.eye(P, dtype=np.float32)
    Ly = np.zeros((P, P), np.float32)
    for m in range(P):
        Ly[m, m] = -1.0
        if m + 1 < P:
            Ly[m + 1, m] = 1.0
    Ey = np.zeros((P, P), np.float32)
    Ey[0, 127] = 1.0
    KyT = np.zeros((P, P), np.float32)
    for m in range(P):
        KyT[m, m] = kk
        if m - 1 >= 0:
            KyT[m - 1, m] = -kk
    KeT = np.zeros((P, P), np.float32)
    KeT[127, 0] = -kk
    mats = {"mID": I, "mLy": Ly, "mEy": Ey, "mNI": -I, "mKp": kk * I,
            "mKn": -kk * I, "mKyT": KyT, "mKeT": KeT}
    return {k: v.astype(np.float16) for k, v in mats.items()}


def _per_core_inputs(f_img, lam_img):
    fa = _to_layout_a(np.asarray(f_img, np.float32))
    u0 = fa.astype(np.float32)                      # W = u
    vb0 = (SIGMA0 * fa).astype(np.float16)
    cfp = (KH * fa).astype(np.float16)

    lamx = np.concatenate([lam_img[1:, :], np.zeros((1, W), np.float32)])
    lx3 = _to_layout_a(lamx).reshape(P, NCH, H).copy()
    lx3[:, :, 255] = 0.0
    lx = lx3.reshape(P, FREE).astype(np.float16)

    lamy = np.concatenate([lam_img[:, 1:], np.zeros((H, 1), np.float32)], axis=1)
    ly = _to_layout_a(lamy).astype(np.float16)  # (c=1,p=127) col already zero

    return {
        "u0": np.ascontiguousarray(u0),
        "vb0": np.ascontiguousarray(vb0),
        "cfp": np.ascontiguousarray(cfp),
        "lx": np.ascontiguousarray(lx),
        "ly": np.ascontiguousarray(ly),
    }


# ---------------------------------------------------------------- custom op
def _register_clip_op():
    """out = clip(in0, -in1, +in1) as a single DVE instruction."""
    from concourse import dve_ops
    from concourse.dve_spec import Spec, Src0, Src1, maxx, minn, lower
    from concourse.dve_uop import DveOpSpec

    for op in dve_ops.OPS:
        if op.name == "TV_CLIP2_ANT":
            return op
    spec = Spec(
        body=minn(maxx(Src0, -Src1), Src1),
        reference=lambda in0, in1, s0, s1, imm2: np.minimum(
            np.maximum(in0, -in1), in1
        ).astype(np.float32),
    )
    op = dve_ops.DveOp("TV_CLIP2_ANT", spec, subdim=False, uops_sha={})
    dve_ops.OPS.append(op)
    dve_ops.CUSTOM_DVE_SPECS[op.name] = spec
    dve_ops._SUB_OPCODE_FOR_NAME[op.name] = (
        max(dve_ops._SUB_OPCODE_FOR_NAME.values()) + 1
    )
    for ver in ("v3", "v4"):
        try:
            s = DveOpSpec(
                name=op.name,
                opcode=dve_ops.get_dve_sub_opcode(op.name),
                uops=lower(spec, ver=ver),
                rd1_en=True,
            )
            op.uops_sha[ver] = s.sha(ver)
        except Exception:
            pass
    return op


CLIP_OP = _register_clip_op()


# ---------------------------------------------------------------- bass build
def split_excess_waits(nc, max_waits=1):
    """This neuronxcc/walrus build encodes at most ONE sync wait per
    instruction; split the excess onto NoOp carriers on the same engine."""
    nsplit = 0
    for f in nc.m.functions:
        for bb in f.blocks:
            il = bb.instructions
            out = []
            for inst in il:
                si = inst.sync_info
                waits = list(si.on_wait) if si and si.on_wait else []
                k = 0
                while len(waits) > max_waits:
                    head, waits = waits[:max_waits], waits[max_waits:]
                    out.append(
                        mybir.InstNoOp(
                            name=f"{inst.name}-waitsplit{k}",
                            engine=inst.engine,
                            ins=[],
                            outs=[],
                            sync_info=mybir.SyncInfo(on_wait=head, on_update=[]),
                        )
                    )
                    k += 1
                    nsplit += 1
                if k:
                    inst.sync_info = mybir.SyncInfo(
                        on_wait=waits,
                        on_update=list(si.on_update) if si.on_update else [],
                    )
                out.append(inst)
            il[:] = out
    return nsplit


USE_FUSED_CLIP = False
FUSE_Q = False


def build_nc(n_iters=N_ITERS, split=True, fused_clip=None):
    if fused_clip is None:
        fused_clip = USE_FUSED_CLIP
    nc = bass.Bass(trn_type="TRN2")

    d_in = {"u0": nc.dram_tensor("u0", [P, FREE], F32, kind="ExternalInput")}
    for name in ("vb0", "cfp", "lx", "ly"):
        d_in[name] = nc.dram_tensor(name, [P, FREE], F16, kind="ExternalInput")
    d_out = nc.dram_tensor("out", [P, FREE], F32, kind="ExternalOutput")
    d_mats = {
        name: nc.inline_tensor(data, name=name)
        for name, data in _make_matrices().items()
    }

    taus, sigs, thetas = _schedule(n_iters)
    with TileContext(nc) as tc:
        with (
            tc.tile_pool(name="state", bufs=1) as state,
            tc.tile_pool(name="scratch", bufs=3) as scratch,
            tc.tile_pool(name="psum", bufs=2, space="PSUM") as psum,
        ):
            # persistent tiles
            Us = [state.tile([P, FREE], F32, name=f"U{i}") for i in range(2)]
            VBs = [state.tile([P, FREE], F16, name=f"VB{i}") for i in range(2)]
            Ps = [state.tile([P, FREE + 2], F16, name=f"Pd{i}") for i in range(2)]
            Qs = [state.tile([P, FREE], F16, name=f"Qd{i}") for i in range(2)]
            LX = state.tile([P, FREE], F16, name="LX")
            LY = state.tile([P, FREE], F16, name="LY")
            NLX = state.tile([P, FREE], F16, name="NLX")
            NLY = state.tile([P, FREE], F16, name="NLY")
            CF = state.tile([P, FREE], F16, name="CF")
            mats = {
                name: state.tile([P, P], F16, name=f"t_{name}") for name in d_mats
            }

            # ---- setup
            nc.sync.dma_start(out=Us[0], in_=d_in["u0"].ap())
            nc.gpsimd.dma_start(out=VBs[0], in_=d_in["vb0"].ap())
            nc.scalar.dma_start(out=CF, in_=d_in["cfp"].ap())
            nc.gpsimd.dma_start(out=LX, in_=d_in["lx"].ap())
            nc.sync.dma_start(out=LY, in_=d_in["ly"].ap())
            for i, name in enumerate(d_mats):
                dma = (nc.gpsimd, nc.sync, nc.scalar)[i % 3]
                dma.dma_start(out=mats[name], in_=d_mats[name].ap())
            nc.vector.memset(Ps[0].bitcast(F32), 0.0)
            nc.vector.memset(Ps[1].bitcast(F32), 0.0)
            nc.vector.memset(Qs[0].bitcast(F32), 0.0)
            if not fused_clip:
                nc.scalar.mul(NLX, LX, -1.0)
                nc.scalar.mul(NLY, LY, -1.0)

            def clip2(out, psrc, L, NL, tag, fuse=None):
                if fused_clip if fuse is None else fuse:
                    L3 = L.rearrange("p (s n) -> p s n", s=1)
                    nc.vector._custom_dve(CLIP_OP, out=out, in0=psrc, in1=L3)
                else:
                    t = scratch.tile([P, FREE], F16, name=f"cl_{tag}", tag=f"cl_{tag}")
                    nc.vector.tensor_tensor(t, psrc, L, AOP.min)
                    nc.vector.tensor_tensor(out, t, NL, AOP.max)

            def mm(out, lhsT, rhs, start, stop):
                nc.tensor.matmul(
                    out, lhsT, rhs, start=start, stop=stop,
                    skip_group_check=True,
                )

            for i in range(n_iters):
                a, b = i % 2, (i + 1) % 2
                Uc, Un = Us[a], Us[b]
                VBc, VBn = VBs[a], VBs[b]
                Pc, Pn = Ps[a], Ps[b]
                Qc, Qn = Qs[a], Qs[b]

                PSq = psum.tile([P, FREE], F32, name="PSq", tag="PSq")
                PSp = psum.tile([P, FREE], F32, name="PSp", tag="PSp")
                PSu = psum.tile([P, FREE], F32, name="PSu", tag="PSu")

                # early matmuls (deps from mid-previous-iteration)
                mm(PSq, mats["mID"], Qc, start=True, stop=False)
                mm(PSp, mats["mID"], Pc[:, 1 : FREE + 1], start=True, stop=False)
                mm(PSu, mats["mID"], CF, start=True, stop=False)

                # on VBc: q-path first (cast readiness beats A2 on ScalarE)
                mm(PSq, mats["mLy"], VBc, start=False, stop=False)
                mm(PSq[:, 0:H], mats["mEy"], VBc[:, H:FREE], start=False,
                   stop=True)
                mm(PSp, mats["mNI"], VBc, start=False, stop=False)
                mm(PSp[:, 0 : H - 1], mats["mID"], VBc[:, 1:H], start=False,
                   stop=False)
                mm(PSp[:, H : FREE - 1], mats["mID"], VBc[:, H + 1 : FREE],
                   start=False, stop=True)

                tau, th = taus[i], thetas[i]
                sig1 = sigs[i + 1]
                c1 = 1.0 / (1.0 + tau)
                s1 = float(c1 * tau / KH)
                s2 = float(sig1 * (1.0 + th) * c1 * tau / KH)
                a1 = float(c1)
                a2 = float(sig1 * ((1.0 + th) * c1 - th))

                # dual clips: P straight from PSUM; Q via ScalarE fp16 cast
                # so both Q ops run in DVE 2x mode
                clip2(Pn[:, 1 : FREE + 1], PSp, LX, NLX, "p")
                Q16 = scratch.tile([P, FREE], F16, name="Q16", tag="Q16")
                nc.scalar.copy(Q16, PSq)
                clip2(Qn, Q16, LY, NLY, "q")
                A2 = scratch.tile([P, FREE], F32, name="A2", tag="A2")
                A1 = scratch.tile([P, FREE], F32, name="A1", tag="A1")
                nc.scalar.mul(A2, Uc, a2)
                nc.scalar.mul(A1, Uc, a1)

                # primal tail
                mm(PSu, mats["mKp"], Pn[:, 1 : FREE + 1], start=False,
                   stop=False)
                mm(PSu, mats["mKn"], Pn[:, 0:FREE], start=False, stop=False)
                mm(PSu, mats["mKyT"], Qn, start=False, stop=False)
                mm(PSu[:, H:FREE], mats["mKeT"], Qn[:, 0:H], start=False,
                   stop=True)

                nc.vector.scalar_tensor_tensor(
                    out=VBn, in0=PSu, scalar=s2, in1=A2,
                    op0=AOP.mult, op1=AOP.add,
                )
                nc.vector.scalar_tensor_tensor(
                    out=Un, in0=PSu, scalar=s1, in1=A1,
                    op0=AOP.mult, op1=AOP.add,
                )

            Ufin = Us[n_iters % 2]
            nc.sync.dma_start(out=d_out.ap(), in_=Ufin)

    nc.finalize()
    if split:
        split_excess_waits(nc)
    return nc


_NC_CACHE = {}


def _get_nc(n_iters=N_ITERS):
    key = n_iters
    if key not in _NC_CACHE:
        _NC_CACHE[key] = build_nc(n_iters)
    return _NC_CACHE[key]


def kernel(f, lam):
    from concourse.bass_utils import run_bass_kernel_spmd

    f = np.asarray(f, dtype=np.float32)
    lam = np.asarray(lam, dtype=np.float32)
    nc = _get_nc()
    in_maps = [_per_core_inputs(f[b], lam[b]) for b in range(B)]
    res = run_bass_kernel_spmd(nc, in_maps, core_ids=list(range(B)))
    return np.stack([_from_layout_a(res.results[b]["out"]) for b in range(B)])


if __name__ == "__main__":
    import sys

    if "--build" in sys.argv:
        import time

        t0 = time.time()
        nc = build_nc(int(sys.argv[sys.argv.index("--build") + 1])
                      if len(sys.argv) > 2 else N_ITERS)
        print(f"build ok in {time.time()-t0:.1f}s")


# revision 23
# speedup vs baseline: 4.0007x; 1.0539x over previous
"""Trainium2 Bass kernel for the DifferentiableTVLayer PDHG solve.

Problem: anisotropic weighted-TV prox via Chambolle-Pock on 8 images of
256x256 (fp32). Sharding: pure data parallel, 1 image per NeuronCore.

The reference runs 200 iterations but is itself ~0.006 rel away from the
converged solution; 120 iterations of the same scheme land within ~0.010
rel of the reference output (tolerance 2e-2), so we run 120.

Per-core layout ("layout A"): tiles [128, 512] where
    tile[p, c*256 + h] = X[h, w],  w = c*128 + p,  c in {0,1}.
W-direction (partition) derivatives are PE matmuls against +-1 operator
matrices; H-direction (free dim) derivatives are PE matmuls against
identity with column-shifted access patterns, accumulated in PSUM.

All PE weights are exactly-representable fp16 constants (+-1 or +-KKH);
all exact scale factors (C1, 2C1-1, 2.0) ride as fp32 immediates in DVE
scalar_tensor_tensor ops, so fp16 only contributes state-quantization
noise (~2e-4), damped by the C1-contraction of the iteration.

Per iteration (state: U fp32; VB, P, Q fp16; bounds LX, LY fp16):
    PSq = I@Qc + Ly@VB + Ey@VB[hi]            (PE, = q + sigma*dy(ubar))
    PSp = I@Pc + (-I)@VB + I@VB<<1            (PE, = p + sigma*dx(ubar))
    PSu = I@CFP + KKH*(dxT(Pn) + dyT(Qn))     (PE, scaled primal residual)
    Pn  = clip(PSp, +-LX); Qn = clip(PSq, +-LY)   (DVE fused custom op)
    Un  = C1*Uc + PSu                          (DVE STT)
    VBn = 2*PSu + A2,  A2 = (2C1-1)*Uc         (DVE STT; A2 on ScalarE)
"""

import numpy as np

import concourse.bass as bass
import concourse.mybir as mybir
from concourse.tile import TileContext

TAU0 = 0.5                  # accelerated CP (Chambolle-Pock Alg.2, mu-strongly convex)
SIGMA0 = 1.0 / (8.0 * TAU0)  # tau*sigma*L^2 = 1 boundary
GAMMA = 0.5
KH = 0.25                    # fixed fp16-exact PE weight for the PSu terms
B, H, W = 8, 256, 256
P, NCH = 128, 2
FREE = NCH * H  # 512
N_ITERS = 38


def _schedule(n):
    """Per-iteration (tau_i, sigma_i, theta_i), plus sigma_{n} lookahead."""
    taus, sigs, thetas = [], [], []
    tau, sig = TAU0, SIGMA0
    for _ in range(n + 1):
        th = 1.0 / float(np.sqrt(1.0 + 2.0 * GAMMA * tau))
        taus.append(tau)
        sigs.append(sig)
        thetas.append(th)
        tau, sig = tau * th, sig / th
    return taus, sigs, thetas

F32 = mybir.dt.float32
F16 = mybir.dt.float16
AOP = mybir.AluOpType


# ---------------------------------------------------------------- host layout
def _to_layout_a(x):
    """[H, W] -> [128, 512]: out[p, c*256+h] = x[h, c*128+p]."""
    return np.ascontiguousarray(
        x.T.reshape(NCH, P, H).transpose(1, 0, 2).reshape(P, FREE)
    )


def _from_layout_a(t):
    return np.ascontiguousarray(
        t.reshape(P, NCH, H).transpose(1, 0, 2).reshape(W, H).T
    )


def _make_matrices():
    """lhsT operator matrices [k, m]: out[m] = sum_k lhsT[k,m] rhs[k]."""
    kk = np.float32(np.float16(KH))
    I = np.eye(P, dtype=np.float32)
    Ly = np.zeros((P, P), np.float32)
    for m in range(P):
        Ly[m, m] = -1.0
        if m + 1 < P:
            Ly[m + 1, m] = 1.0
    Ey = np.zeros((P, P), np.float32)
    Ey[0, 127] = 1.0
    KyT = np.zeros((P, P), np.float32)
    for m in range(P):
        KyT[m, m] = kk
        if m - 1 >= 0:
            KyT[m - 1, m] = -kk
    KeT = np.zeros((P, P), np.float32)
    KeT[127, 0] = -kk
    mats = {"mID": I, "mLy": Ly, "mEy": Ey, "mNI": -I, "mKp": kk * I,
            "mKn": -kk * I, "mKyT": KyT, "mKeT": KeT}
    return {k: v.astype(np.float16) for k, v in mats.items()}


def _per_core_inputs(f_img, lam_img):
    fa = _to_layout_a(np.asarray(f_img, np.float32))
    u0 = fa.astype(np.float32)                      # W = u
    vb0 = (SIGMA0 * fa).astype(np.float16)
    cfp = (KH * fa).astype(np.float16)

    lamx = np.concatenate([lam_img[1:, :], np.zeros((1, W), np.float32)])
    lx3 = _to_layout_a(lamx).reshape(P, NCH, H).copy()
    lx3[:, :, 255] = 0.0
    lx = lx3.reshape(P, FREE).astype(np.float16)

    lamy = np.concatenate([lam_img[:, 1:], np.zeros((H, 1), np.float32)], axis=1)
    ly = _to_layout_a(lamy).astype(np.float16)  # (c=1,p=127) col already zero

    return {
        "u0": np.ascontiguousarray(u0),
        "vb0": np.ascontiguousarray(vb0),
        "cfp": np.ascontiguousarray(cfp),
        "lx": np.ascontiguousarray(lx),
        "ly": np.ascontiguousarray(ly),
    }


# ---------------------------------------------------------------- custom op
def _register_clip_op():
    """out = clip(in0, -in1, +in1) as a single DVE instruction."""
    from concourse import dve_ops
    from concourse.dve_spec import Spec, Src0, Src1, maxx, minn, lower
    from concourse.dve_uop import DveOpSpec

    for op in dve_ops.OPS:
        if op.name == "TV_CLIP2_ANT":
            return op
    spec = Spec(
        body=minn(maxx(Src0, -Src1), Src1),
        reference=lambda in0, in1, s0, s1, imm2: np.minimum(
            np.maximum(in0, -in1), in1
        ).astype(np.float32),
    )
    op = dve_ops.DveOp("TV_CLIP2_ANT", spec, subdim=False, uops_sha={})
    dve_ops.OPS.append(op)
    dve_ops.CUSTOM_DVE_SPECS[op.name] = spec
    dve_ops._SUB_OPCODE_FOR_NAME[op.name] = (
        max(dve_ops._SUB_OPCODE_FOR_NAME.values()) + 1
    )
    for ver in ("v3", "v4"):
        try:
            s = DveOpSpec(
                name=op.name,
                opcode=dve_ops.get_dve_sub_opcode(op.name),
                uops=lower(spec, ver=ver),
                rd1_en=True,
            )
            op.uops_sha[ver] = s.sha(ver)
        except Exception:
            pass
    return op


CLIP_OP = _register_clip_op()


# ---------------------------------------------------------------- bass build
def split_excess_waits(nc, max_waits=1):
    """This neuronxcc/walrus build encodes at most ONE sync wait per
    instruction; split the excess onto NoOp carriers on the same engine."""
    nsplit = 0
    for f in nc.m.functions:
        for bb in f.blocks:
            il = bb.instructions
            out = []
            for inst in il:
                si = inst.sync_info
                waits = list(si.on_wait) if si and si.on_wait else []
                k = 0
                while len(waits) > max_waits:
                    head, waits = waits[:max_waits], waits[max_waits:]
                    out.append(
                        mybir.InstNoOp(
                            name=f"{inst.name}-waitsplit{k}",
                            engine=inst.engine,
                            ins=[],
                            outs=[],
                            sync_info=mybir.SyncInfo(on_wait=head, on_update=[]),
                        )
                    )
                    k += 1
                    nsplit += 1
                if k:
                    inst.sync_info = mybir.SyncInfo(
                        on_wait=waits,
                        on_update=list(si.on_update) if si.on_update else [],
                    )
                out.append(inst)
            il[:] = out
    return nsplit


USE_FUSED_CLIP = False
FUSE_Q = False


def build_nc(n_iters=N_ITERS, split=True, fused_clip=None):
    if fused_clip is None:
        fused_clip = USE_FUSED_CLIP
    nc = bass.Bass(trn_type="TRN2")

    d_in = {"u0": nc.dram_tensor("u0", [P, FREE], F32, kind="ExternalInput")}
    for name in ("vb0", "cfp", "lx", "ly"):
        d_in[name] = nc.dram_tensor(name, [P, FREE], F16, kind="ExternalInput")
    d_out = nc.dram_tensor("out", [P, FREE], F32, kind="ExternalOutput")
    d_mats = {
        name: nc.inline_tensor(data, name=name)
        for name, data in _make_matrices().items()
    }

    taus, sigs, thetas = _schedule(n_iters)
    with TileContext(nc) as tc:
        with (
            tc.tile_pool(name="state", bufs=1) as state,
            tc.tile_pool(name="scratch", bufs=3) as scratch,
            tc.tile_pool(name="psum", bufs=2, space="PSUM") as psum,
        ):
            # persistent tiles
            Us = [state.tile([P, FREE], F32, name=f"U{i}") for i in range(2)]
            VBs = [state.tile([P, FREE], F16, name=f"VB{i}") for i in range(2)]
            Ps = [state.tile([P, FREE + 2], F16, name=f"Pd{i}") for i in range(2)]
            Qs = [state.tile([P, FREE], F16, name=f"Qd{i}") for i in range(2)]
            LX = state.tile([P, FREE], F16, name="LX")
            LY = state.tile([P, FREE], F16, name="LY")
            NLX = state.tile([P, FREE], F16, name="NLX")
            NLY = state.tile([P, FREE], F16, name="NLY")
            CF = state.tile([P, FREE], F16, name="CF")
            mats = {
                name: state.tile([P, P], F16, name=f"t_{name}") for name in d_mats
            }

            # ---- setup
            nc.sync.dma_start(out=Us[0], in_=d_in["u0"].ap())
            nc.gpsimd.dma_start(out=VBs[0], in_=d_in["vb0"].ap())
            nc.scalar.dma_start(out=CF, in_=d_in["cfp"].ap())
            nc.gpsimd.dma_start(out=LX, in_=d_in["lx"].ap())
            nc.sync.dma_start(out=LY, in_=d_in["ly"].ap())
            for i, name in enumerate(d_mats):
                dma = (nc.gpsimd, nc.sync, nc.scalar)[i % 3]
                dma.dma_start(out=mats[name], in_=d_mats[name].ap())
            nc.vector.memset(Ps[0].bitcast(F32), 0.0)
            nc.vector.memset(Ps[1].bitcast(F32), 0.0)
            nc.vector.memset(Qs[0].bitcast(F32), 0.0)
            if not fused_clip:
                nc.scalar.mul(NLX, LX, -1.0)
                nc.scalar.mul(NLY, LY, -1.0)

            def clip2(out, psrc, L, NL, tag, fuse=None):
                if fused_clip if fuse is None else fuse:
                    L3 = L.rearrange("p (s n) -> p s n", s=1)
                    nc.vector._custom_dve(CLIP_OP, out=out, in0=psrc, in1=L3)
                else:
                    t = scratch.tile([P, FREE], F16, name=f"cl_{tag}", tag=f"cl_{tag}")
                    nc.vector.tensor_tensor(t, psrc, L, AOP.min)
                    nc.vector.tensor_tensor(out, t, NL, AOP.max)

            def mm(out, lhsT, rhs, start, stop):
                nc.tensor.matmul(
                    out, lhsT, rhs, start=start, stop=stop,
                    skip_group_check=True,
                )

            for i in range(n_iters):
                a, b = i % 2, (i + 1) % 2
                Uc, Un = Us[a], Us[b]
                VBc, VBn = VBs[a], VBs[b]
                Pc, Pn = Ps[a], Ps[b]
                Qc, Qn = Qs[a], Qs[b]

                PSq = psum.tile([P, FREE], F32, name="PSq", tag="PSq")
                PSp = psum.tile([P, FREE], F32, name="PSp", tag="PSp")
                PSu = psum.tile([P, FREE], F32, name="PSu", tag="PSu")

                # early matmuls (deps from mid-previous-iteration)
                mm(PSq, mats["mID"], Qc, start=True, stop=False)
                mm(PSp, mats["mID"], Pc[:, 1 : FREE + 1], start=True, stop=False)
                mm(PSu, mats["mID"], CF, start=True, stop=False)

                # on VBc: q-path first (cast readiness beats A2 on ScalarE)
                mm(PSq, mats["mLy"], VBc, start=False, stop=False)
                mm(PSq[:, 0:H], mats["mEy"], VBc[:, H:FREE], start=False,
                   stop=True)
                mm(PSp, mats["mNI"], VBc, start=False, stop=False)
                mm(PSp[:, 0 : H - 1], mats["mID"], VBc[:, 1:H], start=False,
                   stop=False)
                mm(PSp[:, H : FREE - 1], mats["mID"], VBc[:, H + 1 : FREE],
                   start=False, stop=True)

                tau, th = taus[i], thetas[i]
                sig1 = sigs[i + 1]
                c1 = 1.0 / (1.0 + tau)
                s1 = float(c1 * tau / KH)
                s2 = float(sig1 * (1.0 + th) * c1 * tau / KH)
                a1 = float(c1)
                a2 = float(sig1 * ((1.0 + th) * c1 - th))

                # dual clips: P straight from PSUM; Q via ScalarE fp16 cast
                # so both Q ops run in DVE 2x mode
                clip2(Pn[:, 1 : FREE + 1], PSp, LX, NLX, "p")
                Q16 = scratch.tile([P, FREE], F16, name="Q16", tag="Q16")
                nc.scalar.copy(Q16, PSq)
                clip2(Qn, Q16, LY, NLY, "q")
                A2 = scratch.tile([P, FREE], F32, name="A2", tag="A2")
                A1 = scratch.tile([P, FREE], F32, name="A1", tag="A1")
                nc.scalar.mul(A2, Uc, a2)
                nc.scalar.mul(A1, Uc, a1)

                # primal tail
                mm(PSu, mats["mKp"], Pn[:, 1 : FREE + 1], start=False,
                   stop=False)
                mm(PSu, mats["mKn"], Pn[:, 0:FREE], start=False, stop=False)
                mm(PSu, mats["mKyT"], Qn, start=False, stop=False)
                mm(PSu[:, H:FREE], mats["mKeT"], Qn[:, 0:H], start=False,
                   stop=True)

                nc.vector.scalar_tensor_tensor(
                    out=VBn, in0=PSu, scalar=s2, in1=A2,
                    op0=AOP.mult, op1=AOP.add,
                )
                nc.vector.scalar_tensor_tensor(
                    out=Un, in0=PSu, scalar=s1, in1=A1,
                    op0=AOP.mult, op1=AOP.add,
                )

            Ufin = Us[n_iters % 2]
            nc.sync.dma_start(out=d_out.ap(), in_=Ufin)

    nc.finalize()
    if split:
        split_excess_waits(nc)
    return nc


_NC_CACHE = {}


def _get_nc(n_iters=N_ITERS):
    key = n_iters
    if key not in _NC_CACHE:
        _NC_CACHE[key] = build_nc(n_iters)
    return _NC_CACHE[key]


def kernel(f, lam):
    from concourse.bass_utils import run_bass_kernel_spmd

    f = np.asarray(f, dtype=np.float32)
    lam = np.asarray(lam, dtype=np.float32)
    nc = _get_nc()
    in_maps = [_per_core_inputs(f[b], lam[b]) for b in range(B)]
    res = run_bass_kernel_spmd(nc, in_maps, core_ids=list(range(B)))
    return np.stack([_from_layout_a(res.results[b]["out"]) for b in range(B)])


if __name__ == "__main__":
    import sys

    if "--build" in sys.argv:
        import time

        t0 = time.time()
        nc = build_nc(int(sys.argv[sys.argv.index("--build") + 1])
                      if len(sys.argv) > 2 else N_ITERS)
        print(f"build ok in {time.time()-t0:.1f}s")


# revision 24
# speedup vs baseline: 4.0443x; 1.0109x over previous
"""Trainium2 Bass kernel for the DifferentiableTVLayer PDHG solve.

Problem: anisotropic weighted-TV prox on 8 images of 256x256 (fp32).
Sharding: pure data parallel, 1 image per NeuronCore (8 cores).

The reference runs 200 plain Chambolle-Pock iterations but is itself
~0.006 rel from the converged solution (tolerance 2e-2). The primal term
0.5||u-f||^2 is 1-strongly convex, so this kernel runs the accelerated
CP schedule (Alg. 2: theta_n = 1/sqrt(1+2*gamma*tau_n), tau->tau*theta,
sigma->sigma/theta) for N_ITERS=38 iterations, landing at ~0.0126 rel.
Host-side numpy does layout/scale prep only.

Per-core layout ("layout A"): tiles [128, 512] where
    tile[p, c*256 + h] = X[h, w],  w = c*128 + p,  c in {0,1}.
W-direction (partition) derivatives are PE matmuls against +-1 operator
matrices; H-direction (free) derivatives are PE matmuls against identity
with column-shifted APs. All adds accumulate in PSUM, so the DVE only
runs 6 ops/iteration. PE weights are exact fp16 (+-1, +-KH); per-iter
scale factors ride as fp32 immediates in STT ops / ScalarE prescales.

Per iteration (W=u fp32; VB=sigma_i*ubar, P, Q fp16; bounds LX/LY fp16):
    PSq = I@Qc + Ly@VB + Ey@VB[hi]       (PE: q + sigma*dy(ubar), PSUM)
    PSp = I@Pc + (-I)@VB + I@VB<<1       (PE: p + sigma*dx(ubar), PSUM)
    Pn  = clip(PSp, +-LX)                (DVE min/max; PSUM->fp16)
    Qn  = clip(cast(PSq), +-LY)          (ScalarE fp16 cast; DVE 2x mode)
    PSu = I@(KH*f) - KH*(dxT Pn + dyT Qn)    (PE, fixed +-KH weights)
    Wn  = s1_i*PSu + A1,  A1 = c1_i*Wc       (DVE STT; ScalarE prescale)
    VBn = s2_i*PSu + A2,  A2 = a2_i*Wc       (DVE STT; ScalarE prescale)
"""

import numpy as np

import concourse.bass as bass
import concourse.mybir as mybir
from concourse.tile import TileContext

TAU0 = 0.5                  # accelerated CP (Chambolle-Pock Alg.2, mu-strongly convex)
SIGMA0 = 1.0 / (8.0 * TAU0)  # tau*sigma*L^2 = 1 boundary
GAMMA = 0.5
KH = 0.25                    # fixed fp16-exact PE weight for the PSu terms
B, H, W = 8, 256, 256
P, NCH = 128, 2
FREE = NCH * H  # 512
N_ITERS = 38


def _schedule(n):
    """Per-iteration (tau_i, sigma_i, theta_i), plus sigma_{n} lookahead."""
    taus, sigs, thetas = [], [], []
    tau, sig = TAU0, SIGMA0
    for _ in range(n + 1):
        th = 1.0 / float(np.sqrt(1.0 + 2.0 * GAMMA * tau))
        taus.append(tau)
        sigs.append(sig)
        thetas.append(th)
        tau, sig = tau * th, sig / th
    return taus, sigs, thetas

F32 = mybir.dt.float32
F16 = mybir.dt.float16
AOP = mybir.AluOpType


# ---------------------------------------------------------------- host layout
def _to_layout_a(x):
    """[H, W] -> [128, 512]: out[p, c*256+h] = x[h, c*128+p]."""
    return np.ascontiguousarray(
        x.T.reshape(NCH, P, H).transpose(1, 0, 2).reshape(P, FREE)
    )


def _from_layout_a(t):
    return np.ascontiguousarray(
        t.reshape(P, NCH, H).transpose(1, 0, 2).reshape(W, H).T
    )


def _make_matrices():
    """lhsT operator matrices [k, m]: out[m] = sum_k lhsT[k,m] rhs[k]."""
    kk = np.float32(np.float16(KH))
    I = np.eye(P, dtype=np.float32)
    Ly = np.zeros((P, P), np.float32)
    for m in range(P):
        Ly[m, m] = -1.0
        if m + 1 < P:
            Ly[m + 1, m] = 1.0
    Ey = np.zeros((P, P), np.float32)
    Ey[0, 127] = 1.0
    KyT = np.zeros((P, P), np.float32)
    for m in range(P):
        KyT[m, m] = kk
        if m - 1 >= 0:
            KyT[m - 1, m] = -kk
    KeT = np.zeros((P, P), np.float32)
    KeT[127, 0] = -kk
    mats = {"mID": I, "mLy": Ly, "mEy": Ey, "mNI": -I, "mKp": kk * I,
            "mKn": -kk * I, "mKyT": KyT, "mKeT": KeT}
    return {k: v.astype(np.float16) for k, v in mats.items()}


def _per_core_inputs(f_img, lam_img):
    fa = _to_layout_a(np.asarray(f_img, np.float32))
    u0 = fa.astype(np.float32)                      # W = u
    vb0 = (SIGMA0 * fa).astype(np.float16)
    cfp = (KH * fa).astype(np.float16)

    lamx = np.concatenate([lam_img[1:, :], np.zeros((1, W), np.float32)])
    lx3 = _to_layout_a(lamx).reshape(P, NCH, H).copy()
    lx3[:, :, 255] = 0.0
    lx = lx3.reshape(P, FREE).astype(np.float16)

    lamy = np.concatenate([lam_img[:, 1:], np.zeros((H, 1), np.float32)], axis=1)
    ly = _to_layout_a(lamy).astype(np.float16)  # (c=1,p=127) col already zero

    return {
        "u0": np.ascontiguousarray(u0),
        "vb0": np.ascontiguousarray(vb0),
        "cfp": np.ascontiguousarray(cfp),
        "lx": np.ascontiguousarray(lx),
        "ly": np.ascontiguousarray(ly),
    }


# ---------------------------------------------------------------- custom op
def _register_clip_op():
    """out = clip(in0, -in1, +in1) as a single DVE instruction."""
    from concourse import dve_ops
    from concourse.dve_spec import Spec, Src0, Src1, maxx, minn, lower
    from concourse.dve_uop import DveOpSpec

    for op in dve_ops.OPS:
        if op.name == "TV_CLIP2_ANT":
            return op
    spec = Spec(
        body=minn(maxx(Src0, -Src1), Src1),
        reference=lambda in0, in1, s0, s1, imm2: np.minimum(
            np.maximum(in0, -in1), in1
        ).astype(np.float32),
    )
    op = dve_ops.DveOp("TV_CLIP2_ANT", spec, subdim=False, uops_sha={})
    dve_ops.OPS.append(op)
    dve_ops.CUSTOM_DVE_SPECS[op.name] = spec
    dve_ops._SUB_OPCODE_FOR_NAME[op.name] = (
        max(dve_ops._SUB_OPCODE_FOR_NAME.values()) + 1
    )
    for ver in ("v3", "v4"):
        try:
            s = DveOpSpec(
                name=op.name,
                opcode=dve_ops.get_dve_sub_opcode(op.name),
                uops=lower(spec, ver=ver),
                rd1_en=True,
            )
            op.uops_sha[ver] = s.sha(ver)
        except Exception:
            pass
    return op


CLIP_OP = _register_clip_op()


# ---------------------------------------------------------------- bass build
def split_excess_waits(nc, max_waits=1):
    """This neuronxcc/walrus build encodes at most ONE sync wait per
    instruction; split the excess onto NoOp carriers on the same engine."""
    nsplit = 0
    for f in nc.m.functions:
        for bb in f.blocks:
            il = bb.instructions
            out = []
            for inst in il:
                si = inst.sync_info
                waits = list(si.on_wait) if si and si.on_wait else []
                k = 0
                while len(waits) > max_waits:
                    head, waits = waits[:max_waits], waits[max_waits:]
                    out.append(
                        mybir.InstNoOp(
                            name=f"{inst.name}-waitsplit{k}",
                            engine=inst.engine,
                            ins=[],
                            outs=[],
                            sync_info=mybir.SyncInfo(on_wait=head, on_update=[]),
                        )
                    )
                    k += 1
                    nsplit += 1
                if k:
                    inst.sync_info = mybir.SyncInfo(
                        on_wait=waits,
                        on_update=list(si.on_update) if si.on_update else [],
                    )
                out.append(inst)
            il[:] = out
    return nsplit


USE_FUSED_CLIP = False
FUSE_Q = False


def build_nc(n_iters=N_ITERS, split=True, fused_clip=None):
    if fused_clip is None:
        fused_clip = USE_FUSED_CLIP
    nc = bass.Bass(trn_type="TRN2")

    d_in = {"u0": nc.dram_tensor("u0", [P, FREE], F32, kind="ExternalInput")}
    for name in ("vb0", "cfp", "lx", "ly"):
        d_in[name] = nc.dram_tensor(name, [P, FREE], F16, kind="ExternalInput")
    d_out = nc.dram_tensor("out", [P, FREE], F32, kind="ExternalOutput")
    d_mats = {
        name: nc.inline_tensor(data, name=name)
        for name, data in _make_matrices().items()
    }

    taus, sigs, thetas = _schedule(n_iters)
    with TileContext(nc) as tc:
        with (
            tc.tile_pool(name="state", bufs=1) as state,
            tc.tile_pool(name="scratch", bufs=3) as scratch,
            tc.tile_pool(name="psum", bufs=2, space="PSUM") as psum,
        ):
            # persistent tiles
            Us = [state.tile([P, FREE], F32, name=f"U{i}") for i in range(2)]
            VBs = [state.tile([P, FREE], F16, name=f"VB{i}") for i in range(2)]
            Ps = [state.tile([P, FREE + 2], F16, name=f"Pd{i}") for i in range(2)]
            Qs = [state.tile([P, FREE], F16, name=f"Qd{i}") for i in range(2)]
            LX = state.tile([P, FREE], F16, name="LX")
            LY = state.tile([P, FREE], F16, name="LY")
            NLX = state.tile([P, FREE], F16, name="NLX")
            NLY = state.tile([P, FREE], F16, name="NLY")
            CF = state.tile([P, FREE], F16, name="CF")
            mats = {
                name: state.tile([P, P], F16, name=f"t_{name}") for name in d_mats
            }

            # ---- setup
            nc.sync.dma_start(out=Us[0], in_=d_in["u0"].ap())
            nc.gpsimd.dma_start(out=VBs[0], in_=d_in["vb0"].ap())
            nc.scalar.dma_start(out=CF, in_=d_in["cfp"].ap())
            nc.gpsimd.dma_start(out=LX, in_=d_in["lx"].ap())
            nc.sync.dma_start(out=LY, in_=d_in["ly"].ap())
            for i, name in enumerate(d_mats):
                dma = (nc.gpsimd, nc.sync, nc.scalar)[i % 3]
                dma.dma_start(out=mats[name], in_=d_mats[name].ap())
            nc.vector.memset(Ps[0].bitcast(F32), 0.0)
            nc.vector.memset(Ps[1].bitcast(F32), 0.0)
            nc.vector.memset(Qs[0].bitcast(F32), 0.0)
            if not fused_clip:
                nc.scalar.mul(NLX, LX, -1.0)
                nc.scalar.mul(NLY, LY, -1.0)

            def clip2(out, psrc, L, NL, tag, fuse=None):
                if fused_clip if fuse is None else fuse:
                    L3 = L.rearrange("p (s n) -> p s n", s=1)
                    nc.vector._custom_dve(CLIP_OP, out=out, in0=psrc, in1=L3)
                else:
                    t = scratch.tile([P, FREE], F16, name=f"cl_{tag}", tag=f"cl_{tag}")
                    nc.vector.tensor_tensor(t, psrc, L, AOP.min)
                    nc.vector.tensor_tensor(out, t, NL, AOP.max)

            def mm(out, lhsT, rhs, start, stop):
                nc.tensor.matmul(
                    out, lhsT, rhs, start=start, stop=stop,
                    skip_group_check=True,
                )

            for i in range(n_iters):
                a, b = i % 2, (i + 1) % 2
                Uc, Un = Us[a], Us[b]
                VBc, VBn = VBs[a], VBs[b]
                Pc, Pn = Ps[a], Ps[b]
                Qc, Qn = Qs[a], Qs[b]

                PSq = psum.tile([P, FREE], F32, name="PSq", tag="PSq")
                PSp = psum.tile([P, FREE], F32, name="PSp", tag="PSp")
                PSu = psum.tile([P, FREE], F32, name="PSu", tag="PSu")

                # early matmuls (deps from mid-previous-iteration)
                mm(PSq, mats["mID"], Qc, start=True, stop=False)
                mm(PSp, mats["mID"], Pc[:, 1 : FREE + 1], start=True, stop=False)
                mm(PSu, mats["mID"], CF, start=True, stop=False)

                # on VBc: q-path first (cast readiness beats A2 on ScalarE)
                mm(PSq, mats["mLy"], VBc, start=False, stop=False)
                mm(PSq[:, 0:H], mats["mEy"], VBc[:, H:FREE], start=False,
                   stop=True)
                mm(PSp, mats["mNI"], VBc, start=False, stop=False)
                mm(PSp[:, 0 : H - 1], mats["mID"], VBc[:, 1:H], start=False,
                   stop=False)
                mm(PSp[:, H : FREE - 1], mats["mID"], VBc[:, H + 1 : FREE],
                   start=False, stop=True)

                tau, th = taus[i], thetas[i]
                sig1 = sigs[i + 1]
                c1 = 1.0 / (1.0 + tau)
                s1 = float(c1 * tau / KH)
                s2 = float(sig1 * (1.0 + th) * c1 * tau / KH)
                a1 = float(c1)
                a2 = float(sig1 * ((1.0 + th) * c1 - th))

                # dual clips: P straight from PSUM; Q via ScalarE fp16 cast
                # so both Q ops run in DVE 2x mode
                clip2(Pn[:, 1 : FREE + 1], PSp, LX, NLX, "p")
                Q16 = scratch.tile([P, FREE], F16, name="Q16", tag="Q16")
                nc.scalar.copy(Q16, PSq)
                clip2(Qn, Q16, LY, NLY, "q")
                A2 = scratch.tile([P, FREE], F32, name="A2", tag="A2")
                A1 = scratch.tile([P, FREE], F32, name="A1", tag="A1")
                nc.scalar.mul(A2, Uc, a2)
                nc.scalar.mul(A1, Uc, a1)

                # primal tail
                mm(PSu, mats["mKp"], Pn[:, 1 : FREE + 1], start=False,
                   stop=False)
                mm(PSu, mats["mKn"], Pn[:, 0:FREE], start=False, stop=False)
                mm(PSu, mats["mKyT"], Qn, start=False, stop=False)
                mm(PSu[:, H:FREE], mats["mKeT"], Qn[:, 0:H], start=False,
                   stop=True)

                nc.vector.scalar_tensor_tensor(
                    out=VBn, in0=PSu, scalar=s2, in1=A2,
                    op0=AOP.mult, op1=AOP.add,
                )
                nc.vector.scalar_tensor_tensor(
                    out=Un, in0=PSu, scalar=s1, in1=A1,
                    op0=AOP.mult, op1=AOP.add,
                )

            Ufin = Us[n_iters % 2]
            nc.sync.dma_start(out=d_out.ap(), in_=Ufin)

    nc.finalize()
    if split:
        split_excess_waits(nc)
    return nc


_NC_CACHE = {}


def _get_nc(n_iters=N_ITERS):
    key = n_iters
    if key not in _NC_CACHE:
        _NC_CACHE[key] = build_nc(n_iters)
    return _NC_CACHE[key]


def kernel(f, lam):
    from concourse.bass_utils import run_bass_kernel_spmd

    f = np.asarray(f, dtype=np.float32)
    lam = np.asarray(lam, dtype=np.float32)
    nc = _get_nc()
    in_maps = [_per_core_inputs(f[b], lam[b]) for b in range(B)]
    res = run_bass_kernel_spmd(nc, in_maps, core_ids=list(range(B)))
    return np.stack([_from_layout_a(res.results[b]["out"]) for b in range(B)])


if __name__ == "__main__":
    import sys

    if "--build" in sys.argv:
        import time

        t0 = time.time()
        nc = build_nc(int(sys.argv[sys.argv.index("--build") + 1])
                      if len(sys.argv) > 2 else N_ITERS)
        print(f"build ok in {time.time()-t0:.1f}s")
